# revision 2
# baseline (speedup 1.0000x reference)
"""Trainium2 Bass kernel for nn_MessageGNN (gnn_message_passing).

Sharding: destination-sharded edges across 8 cores.  Core k owns clauses
[k*50000,(k+1)*50000) and vars [k*12500,(k+1)*12500) and every edge whose
destination falls in its slice, so segment sums are fully core-local.

All 8 cores run ONE identical Bass program (SPMD) dispatched once via
shard_map — per-core variation lives entirely in the data.  The gather
schedule is made uniform by padding every (window, table-chunk) gather
group to the max count over the 8 cores (pad slots gather row 0, carry
zero sat/weight, dst sentinel 1536 so they contribute nothing).

Per core, per edge type:
  - Edges are laid out window-major (1024 destinations per window),
    bucketed by 32768-row gather-table chunk (int16 index limit of
    dma_gather) and sorted by destination.  x^T tiles arrive
    feature-major from fp16 transpose-mode dma_gather.
  - Edge MLP per 128-edge tile: stationary x^T / sat^T against moving
    weight chunks accumulate z[e,d] into a grouped PSUM tile; one Prelu
    (alpha=0.1) activation per 4-tile group does the leaky relu.
  - Segment-mean via one-hot matmul over the full 1024-dst window (one
    DVE op builds onehot * (1/cnt); two N=512 matmuls accumulate the
    window's h^T halves in PSUM).
  - Node MLP fused per window: 4 weight-chunk matmuls (feats+bias / h /
    ctx / emb) with the ctx gather folded into a host-computed
    projection driven by a one-hot.  Phase-3 partial sums accumulate
    into a [128,64] tile per node type; the 64-row ctx update finishes
    on host.
"""

import sys

sys.path.insert(0, "/opt/trn_rl_repo")

import numpy as np

NV, NC, NU, E, D = 100000, 400000, 64, 1200000, 128
M = 8
WIN = 1024
CHUNK = 32768
PAD_DST = 1536.0
P = 128
GRP = 4  # tiles per grouped-Prelu PSUM tile

F16 = np.float16
F32 = np.float32


def _wrap_idx(vals):
    n = len(vals)
    arr = np.zeros((16, n // 16), np.int16)
    if n:
        arr[np.arange(n) % 16, np.arange(n) // 16] = vals
    return np.tile(arr, (8, 1))


def _side_plan(src, dst, n_dst, tab_rows):
    """Uniform cross-core schedule for one edge type.

    Returns (sched, S, total_icols, percore) where sched is shared by all
    cores and percore[k] holds core k's sorted edge arrays."""
    nwin = (n_dst + WIN - 1) // WIN
    nchunk = (tab_rows + CHUNK - 1) // CHUNK
    counts = np.zeros((M, nwin, nchunk), np.int64)
    percore = []
    for k in range(M):
        base = k * n_dst
        mask = (dst >= base) & (dst < base + n_dst)
        es = np.nonzero(mask)[0]
        dstl = dst[es] - base
        srcl = src[es]
        w_id = dstl // WIN
        c_id = srcl // CHUNK
        order = np.lexsort((dstl, c_id, w_id))
        es, dstl, srcl, w_id, c_id = (a[order] for a in (es, dstl, srcl, w_id, c_id))
        np.add.at(counts[k], (w_id, c_id), 1)
        percore.append((es, dstl, srcl, w_id, c_id))
    npad = ((counts.max(0) + P - 1) // P) * P  # [nwin, nchunk]

    sched = []
    icol = 0
    for w in range(nwin):
        groups = []
        off = 0
        for c in range(nchunk):
            n = int(npad[w, c])
            if n == 0:
                continue
            groups.append(dict(chunk=c, n=n, off=off, icol=icol))
            off += n
            icol += n // 16
        sched.append(dict(slots=off, groups=groups))
    S = sum(wm["slots"] for wm in sched)
    if S == 0:
        sched[0] = dict(slots=P, groups=[dict(chunk=0, n=P, off=0, icol=0)])
        S, icol = P, P // 16
    return dict(sched=sched, S=S, icols=icol, nwin=nwin, nchunk=nchunk), percore


def _fill_side(plan, edges, sat, we, dst_glob):
    """Core-local slot arrays laid out per the shared schedule."""
    sched, S, icols, nchunk = plan["sched"], plan["S"], plan["icols"], plan["nchunk"]
    es, dstl, srcl, w_id, c_id = edges
    key = w_id * nchunk + c_id  # non-decreasing after the lexsort

    slot_src = np.zeros(S, np.int64)
    slot_dstw = np.full(S, -1, np.int64)
    slot_e = np.full(S, -1, np.int64)
    idxA = np.zeros((P, icols), np.int16)
    base = 0
    for w, wm in enumerate(sched):
        for g in wm["groups"]:
            c = g["chunk"]
            lo = np.searchsorted(key, w * nchunk + c, "left")
            hi = np.searchsorted(key, w * nchunk + c, "right")
            n = hi - lo
            s0 = base + g["off"]
            loc = np.zeros(g["n"], np.int64)
            loc[:n] = srcl[lo:hi] - c * CHUNK
            slot_src[s0:s0 + g["n"]] = loc
            slot_dstw[s0:s0 + n] = dstl[lo:hi] - w * WIN
            slot_e[s0:s0 + n] = es[lo:hi]
            idxA[:, g["icol"]:g["icol"] + g["n"] // 16] = _wrap_idx(loc)
        base += wm["slots"]

    dst_rel = np.where(slot_dstw >= 0, slot_dstw, int(PAD_DST)).astype(F32)
    real = slot_e >= 0
    wslot = np.zeros(S, F32)
    wslot[real] = we[dst_glob[slot_e[real]]]
    satA = np.zeros((5, S), F16)
    satA[:4, real] = sat[slot_e[real]].T.astype(F16)
    satA[4, real] = 1.0
    return dict(
        idxA=idxA,
        dstA=np.ascontiguousarray(dst_rel.reshape(S // P, P).T.astype(F32)),
        wA=np.ascontiguousarray(wslot.reshape(S // P, P).T),
        satA=satA,
    )


def _node_prep(feats, emb, ctx_ids, n_nodes, nwin):
    Np = nwin * WIN
    fT = np.zeros((feats.shape[1] + 1, Np), F16)
    fT[:-1, :n_nodes] = feats.T.astype(F16)
    fT[-1, :n_nodes] = 1.0
    cx = np.full(Np, 300.0, F32)
    cx[:n_nodes] = ctx_ids.astype(F32)
    cxT = np.ascontiguousarray(cx.reshape(Np // P, P).T.astype(F16))
    # transposed emb for the groups past the last full 512-node boundary
    tail0 = (n_nodes // 512) * 512
    n_tail = max((Np - tail0) // 512, 1)
    tailT = np.zeros((P, 512 * n_tail), F16)
    if n_nodes > tail0:
        tailT[:, :n_nodes - tail0] = emb[tail0:].T.astype(F16)
    return fT, cxT, tailT


def _build_program(meta, skip=()):
    import concourse.mybir as mybir
    import concourse.tile as tile
    from concourse import bacc
    from concourse.masks import make_identity

    f16, f32, i16, i32 = (mybir.dt.float16, mybir.dt.float32,
                          mybir.dt.int16, mybir.dt.int32)
    cs, vs = meta["CS"], meta["VS"]

    nc = bacc.Bacc("TRN2", target_bir_lowering=False, debug=False, num_devices=1)
    io = {}

    def dram(name, shape, dt, kind="ExternalInput"):
        io[name] = nc.dram_tensor(name, list(shape), dt, kind=kind)
        return io[name]

    for side in ("A", "B"):
        plan = meta[side]
        dram(f"gtab{side}", [meta["tabrows"][side], D], f16)
        dram(f"idx{side}", [P, plan["icols"]], i16)
        dram(f"dst{side}", [P, plan["S"] // P], f32)
        dram(f"w{side}", [P, plan["S"] // P], f32)
        dram(f"sat{side}", [5, plan["S"]], f16)
        dram(f"Wemb{side}", [P, D], f16)
        dram(f"Wsat{side}", [5, D], f16)
    for sd, n_nodes in (("C", cs), ("V", vs)):
        Np = meta[f"Np{sd}"]
        dram(f"featsT{sd}", [17, Np], f16)
        dram(f"embrows{sd}", [n_nodes, D], f16)
        n_tail = (Np - (n_nodes // 512) * 512) // 512
        dram(f"embtail{sd}", [P, 512 * max(n_tail, 1)], f16)
        dram(f"ctx{sd}", [P, Np // P], f16)
        dram(f"Wf{sd}", [17, D], f16)
        dram(f"Wh{sd}", [P, D], f16)
        dram(f"We{sd}", [P, D], f16)
        dram(f"ctxproj{sd}", [64, D], f16)
    dram("outC", [cs, D], f16, kind="ExternalOutput")
    dram("outV", [vs, D], f16, kind="ExternalOutput")
    dram("accC", [P, 64], f32, kind="ExternalOutput")
    dram("accV", [P, 64], f32, kind="ExternalOutput")

    stage_max = max(
        max((wm["slots"] for wm in meta["A"]["sched"]), default=P),
        max((wm["slots"] for wm in meta["B"]["sched"]), default=P),
        P,
    )
    idx_max = max(
        max((g["n"] // 16 for plan in (meta["A"], meta["B"])
             for wm in plan["sched"] for g in wm["groups"]), default=8),
        8,
    )

    with tile.TileContext(nc) as tc:
        with tc.tile_pool(name="const", bufs=1) as cpool, \
             tc.tile_pool(name="stage", bufs=2) as spool, \
             tc.tile_pool(name="work", bufs=2) as wpool, \
             tc.tile_pool(name="hbuf", bufs=2) as hpool, \
             tc.tile_pool(name="psA", bufs=2, space="PSUM") as psA, \
             tc.tile_pool(name="psH", bufs=1, space="PSUM") as psH, \
             tc.tile_pool(name="psN", bufs=1, space="PSUM") as psN:

            ident = cpool.tile([P, P], f32)
            make_identity(nc, ident[:])
            identF = cpool.tile([P, P], f16)
            nc.vector.tensor_copy(identF[:], ident[:])
            iota_i = cpool.tile([P, WIN], i32)
            nc.gpsimd.iota(iota_i[:], pattern=[[1, WIN]], base=0, channel_multiplier=0)
            iota16 = cpool.tile([P, WIN], f16)
            nc.vector.tensor_copy(iota16[:], iota_i[:])
            iota64f = cpool.tile([P, 64], f32)
            nc.vector.tensor_copy(iota64f[:], iota_i[:, :64])
            z1 = cpool.tile([1, P], f16)
            nc.gpsimd.memset(z1[:], 0.0)
            z512 = cpool.tile([1, 512], f16)
            nc.gpsimd.memset(z512[:], 0.0)

            wt = {}
            for nm in ("WembA", "WsatA", "WembB", "WsatB",
                       "WfC", "WhC", "WeC", "ctxprojC",
                       "WfV", "WhV", "WeV", "ctxprojV"):
                t = cpool.tile(list(io[nm].shape), f16, tag=nm)
                nc.sync.dma_start(t[:], io[nm][:])
                wt[nm] = t

            acc_sb = {}
            for sd in ("C", "V"):
                a = cpool.tile([P, 64], f32, tag=f"acc{sd}")
                nc.vector.memset(a[:], 0.0)
                acc_sb[sd] = a

            for side, sd, n_nodes in (("A", "C", cs), ("B", "V", vs)):
                plan = meta[side]
                gtab = io[f"gtab{side}"]
                tabrows = meta["tabrows"][side]
                tile_off = 0
                for w, wm in enumerate(plan["sched"]):
                    slots = wm["slots"]
                    ntiles = slots // P
                    stage = spool.tile([P, 1, stage_max], f16, tag="stage")
                    for g in (wm["groups"] if "gather" not in skip else []):
                        n = g["n"]
                        it = wpool.tile([P, idx_max], i16, tag="idx")
                        nc.sync.dma_start(
                            it[:, :n // 16],
                            io[f"idx{side}"][:, g["icol"]:g["icol"] + n // 16])
                        c0 = g["chunk"] * CHUNK
                        c1 = min(c0 + CHUNK, tabrows)
                        # >512-idx transpose gathers crash the exec unit;
                        # split into <=512-idx calls (wrap layout slices
                        # cleanly at 512 = 32 idx columns)
                        for o in range(0, n, 512):
                            ns = min(512, n - o)
                            nc.gpsimd.dma_gather(
                                out_ap=stage[:, :, g["off"] + o:g["off"] + o + ns],
                                in_ap=gtab[c0:c1, :],
                                idxs_ap=it[:, o // 16:o // 16 + ns // 16],
                                num_idxs=ns, num_idxs_reg=ns, elem_size=D,
                                transpose=True)
                    if ntiles:
                        dstt = wpool.tile([P, max(ntiles, 1)], f32, tag="dstt")
                        nc.sync.dma_start(dstt[:, :ntiles],
                                          io[f"dst{side}"][:, tile_off:tile_off + ntiles])
                        wtt = wpool.tile([P, max(ntiles, 1)], f32, tag="wtt")
                        nc.sync.dma_start(wtt[:, :ntiles],
                                          io[f"w{side}"][:, tile_off:tile_off + ntiles])
                        satt = wpool.tile([5, stage_max], f16, tag="satt")
                        nc.sync.dma_start(
                            satt[:, :slots],
                            io[f"sat{side}"][:, tile_off * P:tile_off * P + slots])
                    hps = [psH.tile([P, 512], f32, tag=f"h{i}", name=f"hps{i}")
                           for i in range(2)]
                    for i in range(2):
                        nc.tensor.matmul(hps[i][:], lhsT=z1[:], rhs=z512[:],
                                         start=True, stop=False,
                                         skip_group_check=True)
                    for t0g in (range(0, ntiles, GRP) if "tiles" not in skip else []):
                        gn = min(GRP, ntiles - t0g)
                        mps = psA.tile([P, GRP * P], f32, tag="mps")
                        for j in range(gn):
                            t = t0g + j
                            nc.tensor.matmul(mps[:, j * P:(j + 1) * P],
                                             lhsT=stage[:, 0, t * P:(t + 1) * P],
                                             rhs=wt[f"Wemb{side}"][:],
                                             start=True, stop=False)
                            nc.tensor.matmul(mps[:, j * P:(j + 1) * P],
                                             lhsT=satt[:, t * P:(t + 1) * P],
                                             rhs=wt[f"Wsat{side}"][:],
                                             start=False, stop=True)
                        msb = wpool.tile([P, GRP * P], f16, tag="msb")
                        nc.scalar.activation(msb[:, :gn * P], mps[:, :gn * P],
                                             mybir.ActivationFunctionType.Prelu,
                                             alpha=0.1)
                        for j in range(gn):
                            t = t0g + j
                            ohw = wpool.tile([P, WIN], f16, tag="ohw")
                            nc.vector.scalar_tensor_tensor(
                                out=ohw[:], in0=iota16[:],
                                scalar=dstt[:, t:t + 1],
                                in1=wtt[:, t:t + 1].to_broadcast([P, WIN]),
                                op0=mybir.AluOpType.is_equal,
                                op1=mybir.AluOpType.mult)
                            for half in range(2):
                                nc.tensor.matmul(
                                    hps[half][:], lhsT=msb[:, j * P:(j + 1) * P],
                                    rhs=ohw[:, half * 512:(half + 1) * 512],
                                    start=False, stop=True, skip_group_check=True)
                    tile_off += ntiles
                    hT = hpool.tile([P, WIN], f16, tag="hT")
                    nc.vector.tensor_copy(hT[:, :512], hps[0][:])
                    nc.vector.tensor_copy(hT[:, 512:], hps[1][:])

                    # ---- node phase for this window (WIN nodes, padded) ----
                    for g0 in ((0, 512) if "node" not in skip else ()):
                        cga = w * WIN + g0
                        ctx16 = wpool.tile([P, 4], f16, tag="ctx16")
                        nc.sync.dma_start(ctx16[:], io[f"ctx{sd}"][:, cga // P:cga // P + 4])
                        ctx32 = wpool.tile([P, 4], f32, tag="ctx32")
                        nc.vector.tensor_copy(ctx32[:], ctx16[:])
                        featsl = wpool.tile([17, 512], f16, tag="featsl")
                        nc.sync.dma_start(featsl[:], io[f"featsT{sd}"][:, cga:cga + 512])
                        embl = wpool.tile([P, 512], f16, tag="embl")
                        if cga + 512 <= n_nodes:
                            nc.sync.dma_start(embl[:],
                                              io[f"embrows{sd}"][cga:cga + 512, :],
                                              transpose=True)
                        else:
                            tcol = cga - (n_nodes // 512) * 512
                            nc.sync.dma_start(embl[:],
                                              io[f"embtail{sd}"][:, tcol:tcol + 512])
                        ohuT = wpool.tile([64, 512], f16, tag="ohuT")
                        ohu_f = []
                        for j in range(4):
                            ohuf = wpool.tile([P, 64], f16, tag=f"ohuf{j}")
                            nc.vector.tensor_single_scalar(
                                out=ohuf[:], in_=iota64f[:],
                                scalar=ctx32[:, j:j + 1], op=mybir.AluOpType.is_equal)
                            ohu_f.append(ohuf)
                            tps = psA.tile([P, P], f16, tag="tp")
                            nc.tensor.matmul(tps[:64, :], lhsT=ohuf[:], rhs=identF[:],
                                             is_transpose=True, skip_group_check=True)
                            nc.vector.tensor_copy(ohuT[:, j * P:(j + 1) * P], tps[:64, :])
                        nps = psN.tile([P, 512], f32, tag="nps")
                        nc.tensor.matmul(nps[:], lhsT=wt[f"Wf{sd}"][:],
                                         rhs=featsl[:], start=True, stop=False)
                        nc.tensor.matmul(nps[:], lhsT=wt[f"Wh{sd}"][:],
                                         rhs=hT[:, g0:g0 + 512], start=False, stop=False)
                        nc.tensor.matmul(nps[:], lhsT=wt[f"ctxproj{sd}"][:],
                                         rhs=ohuT[:], start=False, stop=False)
                        nc.tensor.matmul(nps[:], lhsT=wt[f"We{sd}"][:],
                                         rhs=embl[:], start=False, stop=True)
                        nsb = wpool.tile([P, 512], f16, tag="nsb")
                        nc.scalar.activation(nsb[:], nps[:],
                                             mybir.ActivationFunctionType.Prelu,
                                             alpha=0.1)
                        aps = psN.tile([P, 64], f32, tag="aps")
                        for j in range(4):
                            rows = min(P, max(0, n_nodes - (cga + j * P)))
                            tps2 = psA.tile([P, P], f16, tag="tp")
                            nc.tensor.matmul(tps2[:], lhsT=nsb[:, j * P:(j + 1) * P],
                                             rhs=identF[:], is_transpose=True,
                                             skip_group_check=True)
                            osb = wpool.tile([P, P], f16, tag="osb")
                            nc.vector.tensor_copy(osb[:], tps2[:])
                            if rows > 0:
                                out_t = io["outC"] if sd == "C" else io["outV"]
                                nc.sync.dma_start(
                                    out_t[cga + j * P:cga + j * P + rows, :],
                                    osb[:rows, :])
                            nc.tensor.matmul(aps[:], lhsT=osb[:], rhs=ohu_f[j][:],
                                             start=(j == 0), stop=(j == 3))
                        nc.vector.tensor_add(acc_sb[sd][:], acc_sb[sd][:], aps[:])

            nc.sync.dma_start(io["accC"][:], acc_sb["C"][:])
            nc.sync.dma_start(io["accV"][:], acc_sb["V"][:])
    nc.compile()
    return nc


_spmd_state = {}

REPLICATED = ("gtabA", "gtabB", "WembA", "WsatA", "WembB", "WsatB",
              "WfC", "WhC", "WeC", "ctxprojC", "WfV", "WhV", "WeV", "ctxprojV")


def _run_spmd(nc, per_core_maps, repl_map):
    """One shard_map dispatch running the identical program on all 8 cores."""
    import concourse.mybir as mybir
    import jax
    from concourse.bass2jax import (_bass_exec_p, install_neuronx_cc_hook,
                                    partition_id_tensor)
    from jax.experimental.shard_map import shard_map
    from jax.sharding import Mesh, NamedSharding, PartitionSpec

    install_neuronx_cc_hook()
    partition_name = nc.partition_id_tensor.name if nc.partition_id_tensor else None
    in_names, out_names, out_avals, zero_shapes = [], [], [], []
    for alloc in nc.m.functions[0].allocations:
        if not isinstance(alloc, mybir.MemoryLocationSet):
            continue
        name = alloc.memorylocations[0].name
        if alloc.kind == "ExternalInput":
            if name != partition_name:
                in_names.append(name)
        elif alloc.kind == "ExternalOutput":
            shape = tuple(alloc.tensor_shape)
            dtype = mybir.dt.np(alloc.dtype)
            out_names.append(name)
            out_avals.append(jax.core.ShapedArray(shape, dtype))
            zero_shapes.append((shape, dtype))
    n_params = len(in_names)
    n_outs = len(out_names)
    all_names = list(in_names) + list(out_names)
    if partition_name is not None:
        all_names.append(partition_name)
    donate = tuple(range(n_params, n_params + n_outs))

    def _body(*args):
        operands = list(args)
        if partition_name is not None:
            operands.append(partition_id_tensor())
        return tuple(_bass_exec_p.bind(
            *operands, out_avals=tuple(out_avals), in_names=tuple(all_names),
            out_names=tuple(out_names), lowering_input_output_aliases=(),
            sim_require_finite=True, sim_require_nnan=True, nc=nc))

    devices = jax.devices()[:M]
    mesh = Mesh(np.asarray(devices), ("core",))
    in_specs = tuple(
        PartitionSpec() if nm in REPLICATED else PartitionSpec("core")
        for nm in in_names) + (PartitionSpec("core"),) * n_outs
    out_specs = (PartitionSpec("core"),) * n_outs
    sharded = jax.jit(
        shard_map(_body, mesh=mesh, in_specs=in_specs, out_specs=out_specs,
                  check_rep=False),
        donate_argnums=donate, keep_unused=True)

    sh_core = NamedSharding(mesh, PartitionSpec("core"))
    sh_repl = NamedSharding(mesh, PartitionSpec())

    # Big replicated tables ship sharded (1x over the wire) and are
    # broadcast device-side by an all_gather; small ones ship replicated.
    def _ag(x):
        return jax.lax.all_gather(x, "core", axis=0, tiled=True)

    ag_fn = jax.jit(
        shard_map(_ag, mesh=mesh, in_specs=(PartitionSpec("core"),),
                  out_specs=PartitionSpec(), check_rep=False),
        out_shardings=sh_repl)

    # The gather tables ship sharded once; core k's shard doubles as its
    # emb-rows input for the node phase (embrowsC = clause rows = gtabB
    # shard, embrowsV = var rows = gtabA shard).
    shard_alias = {"embrowsC": "gtabB", "embrowsV": "gtabA"}
    shard_handles = {
        gnm: jax.device_put(repl_map[gnm], sh_core)
        for gnm in ("gtabA", "gtabB") if gnm in repl_map
    }
    d_ins = []
    for nm in in_names:
        if nm in shard_alias:
            d_ins.append(shard_handles[shard_alias[nm]])
        elif nm in shard_handles:
            d_ins.append(ag_fn(shard_handles[nm]))
        elif nm in REPLICATED:
            d_ins.append(jax.device_put(repl_map[nm], sh_repl))
        else:
            d_ins.append(jax.device_put(
                np.concatenate([m[nm] for m in per_core_maps], 0), sh_core))
    d_zeros = [jax.device_put(np.zeros((M * s[0], *s[1:]), dt), sh_core)
               for (s, dt) in zero_shapes]

    outs = sharded(*d_ins, *d_zeros)
    host = {nm: np.asarray(o) for nm, o in zip(out_names, outs)}

    _spmd_state.clear()
    _spmd_state.update(fn=sharded, d_ins=d_ins, outs=list(outs),
                       out_names=out_names, sh_core=sh_core,
                       zero_shapes=zero_shapes, nc=nc)
    return host


def _redispatch():
    """Re-run the compiled program once (fresh zero output buffers)."""
    import jax
    st = _spmd_state
    d_zeros = [jax.device_put(np.zeros((M * s[0], *s[1:]), dt), st["sh_core"])
               for (s, dt) in st["zero_shapes"]]
    outs = st["fn"](*st["d_ins"], *d_zeros)
    st["outs"] = list(outs)
    return {nm: np.asarray(o) for nm, o in zip(st["out_names"], outs)}


def _spot_check(host, inp, a_src, a_dst, c_src, c_dst, we_c, we_v,
                clause_ctx, var_ctx, n_sample=48, seed=7):
    """Exact host recomputation of a few output nodes.  Clean device runs
    differ by ~2e-3 absmax (f16 path); the terminal's silent corruptions
    differ by ~2.5 — threshold 0.15 separates them cleanly."""
    rng = np.random.default_rng(seed)
    lrelu = lambda x: np.where(x >= 0, x, 0.1 * x)
    ok = True
    for (emb_src, src, dst, sat, Wm, bm, feats, Wn, bn, ctx_ids, emb_self,
         outk, n_nodes) in (
            (inp["var_emb"], a_src, a_dst, inp["edge_sat_vc"], inp["W_vc"],
             inp["b_vc"], inp["clause_feats"], inp["W_c"], inp["b_c"],
             clause_ctx, inp["clause_emb"], "outC", NC),
            (inp["clause_emb"], c_src, c_dst, inp["edge_sat_cv"], inp["W_cv"],
             inp["b_cv"], inp["var_feats"], inp["W_v"], inp["b_v"],
             var_ctx, inp["var_emb"], "outV", NV)):
        n_nodes = feats.shape[0]
        sample = rng.choice(n_nodes, size=min(n_sample, n_nodes), replace=False)
        sel = np.isin(dst, sample)
        es, ds = src[sel], dst[sel]
        m = lrelu(np.concatenate([sat[sel],
                                  emb_src[es].astype(F32)], 1)
                  @ Wm.astype(F32) + bm.astype(F32))
        h = np.zeros((n_nodes, D), F32)
        np.add.at(h, ds, m)
        cnt = np.bincount(dst, minlength=n_nodes).astype(F32)
        h = h / np.maximum(cnt, 1.0)[:, None]
        ctx_e = inp["ctx_emb"][ctx_ids[sample]].astype(F32)
        z = np.concatenate([feats[sample].astype(F32), h[sample], ctx_e,
                            emb_self[sample].astype(F32)], 1) \
            @ Wn.astype(F32) + bn.astype(F32)
        ref = lrelu(z)
        got = host[outk].astype(F32)[sample]
        if np.abs(got - ref).max() > 0.15:
            ok = False
    return ok


def kernel(**inputs):
    inp = {k: np.asarray(v) for k, v in inputs.items()}
    var_emb, clause_emb, ctx_emb = inp["var_emb"], inp["clause_emb"], inp["ctx_emb"]
    nv, ncl, nu = var_emb.shape[0], clause_emb.shape[0], ctx_emb.shape[0]
    cs, vs = ncl // M, nv // M

    W_vc, b_vc = inp["W_vc"].astype(F32), inp["b_vc"].astype(F32)
    W_cv, b_cv = inp["W_cv"].astype(F32), inp["b_cv"].astype(F32)
    W_c, b_c = inp["W_c"].astype(F32), inp["b_c"].astype(F32)
    W_v, b_v = inp["W_v"].astype(F32), inp["b_v"].astype(F32)

    a_src = inp["assigns_src"].astype(np.int64)
    a_dst = inp["assigns_dst"].astype(np.int64)
    c_src = inp["contains_src"].astype(np.int64)
    c_dst = inp["contains_dst"].astype(np.int64)
    var_ctx = inp["var_ctx"].astype(np.int64)
    clause_ctx = inp["clause_ctx"].astype(np.int64)

    cnt_c = np.bincount(a_dst, minlength=ncl).astype(F32)
    cnt_v = np.bincount(c_dst, minlength=nv).astype(F32)
    we_c = 1.0 / np.maximum(cnt_c, 1.0)
    we_v = 1.0 / np.maximum(cnt_v, 1.0)

    gtabA = var_emb.astype(F16)      # assigns gathers var_emb
    gtabB = clause_emb.astype(F16)   # contains gathers clause_emb

    # edge MLP weight chunks (+bias row on the sat chunk)
    WembA = np.ascontiguousarray(W_vc[4:4 + D]).astype(F16)
    WsatA = np.vstack([W_vc[:4], b_vc[None, :]]).astype(F16)
    WembB = np.ascontiguousarray(W_cv[4:4 + D]).astype(F16)
    WsatB = np.vstack([W_cv[:4], b_cv[None, :]]).astype(F16)

    # node MLP chunks: rows [0:16 feats][16:144 h][144:272 ctx][272:400 emb]
    def node_w(Wn, bn):
        nf = Wn.shape[0] - 3 * D
        Wf = np.vstack([Wn[:nf], bn[None, :]]).astype(F16)
        Wh = np.ascontiguousarray(Wn[nf:nf + D]).astype(F16)
        ctxproj = (ctx_emb.astype(F32) @ Wn[nf + D:nf + 2 * D]).astype(F16)
        We = np.ascontiguousarray(Wn[nf + 2 * D:nf + 3 * D]).astype(F16)
        return Wf, Wh, ctxproj, We

    WfC, WhC, ctxprojC, WeC = node_w(W_c, b_c)
    WfV, WhV, ctxprojV, WeV = node_w(W_v, b_v)

    planA, edgesA = _side_plan(a_src, a_dst, cs, nv)
    planB, edgesB = _side_plan(c_src, c_dst, vs, ncl)
    nwinC, nwinV = planA["nwin"], planB["nwin"]

    per_core_maps = []
    for k in range(M):
        mA = _fill_side(planA, edgesA[k], inp["edge_sat_vc"], we_c, a_dst)
        mB = _fill_side(planB, edgesB[k], inp["edge_sat_cv"], we_v, c_dst)
        fTC, cxC, tailC = _node_prep(inp["clause_feats"][k * cs:(k + 1) * cs],
                                     clause_emb[k * cs:(k + 1) * cs],
                                     clause_ctx[k * cs:(k + 1) * cs], cs, nwinC)
        fTV, cxV, tailV = _node_prep(inp["var_feats"][k * vs:(k + 1) * vs],
                                     var_emb[k * vs:(k + 1) * vs],
                                     var_ctx[k * vs:(k + 1) * vs], vs, nwinV)
        per_core_maps.append(dict(
            idxA=mA["idxA"], dstA=mA["dstA"], wA=mA["wA"], satA=mA["satA"],
            idxB=mB["idxA"], dstB=mB["dstA"], wB=mB["wA"], satB=mB["satA"],
            featsTC=fTC, embtailC=tailC, ctxC=cxC,
            featsTV=fTV, embtailV=tailV, ctxV=cxV,
        ))

    repl_map = dict(
        gtabA=gtabA, gtabB=gtabB,
        WembA=WembA, WsatA=WsatA, WembB=WembB, WsatB=WsatB,
        WfC=WfC, WhC=WhC, WeC=WeC, ctxprojC=ctxprojC,
        WfV=WfV, WhV=WhV, WeV=WeV, ctxprojV=ctxprojV,
    )

    meta = dict(A=planA, B=planB, NpC=nwinC * WIN, NpV=nwinV * WIN,
                CS=cs, VS=vs, tabrows=dict(A=nv, B=ncl))
    nc = _build_program(meta)
    host = _run_spmd(nc, per_core_maps, repl_map)
    # the terminal occasionally corrupts results silently; verify a node
    # sample against exact host math and re-dispatch once if it trips
    for _ in range(2):
        if _spot_check(host, inp, a_src, a_dst, c_src, c_dst, we_c, we_v,
                       clause_ctx, var_ctx):
            break
        host = _redispatch()

    new_clause = host["outC"].astype(F32)           # [8*cs, D] in core order
    new_var = host["outV"].astype(F32)              # [8*vs, D]
    accC = host["accC"].reshape(M, P, 64).sum(0)    # [128 d, 64 u]
    accV = host["accV"].reshape(M, P, 64).sum(0)

    cnt_cu = np.bincount(clause_ctx, minlength=nu).astype(F32)
    cnt_vu = np.bincount(var_ctx, minlength=nu).astype(F32)
    c_ctx = (accC / np.maximum(cnt_cu, 1.0)[None, :]).T   # [64, 128]
    v_ctx = (accV / np.maximum(cnt_vu, 1.0)[None, :]).T
    zu = np.concatenate([inp["ctx_feats"].astype(F32), c_ctx, v_ctx,
                         ctx_emb.astype(F32)], 1) @ inp["W_u"].astype(F32) \
        + inp["b_u"].astype(F32)
    new_ctx = np.where(zu >= 0, zu, 0.1 * zu).astype(F32)

    return np.concatenate([new_clause, new_var, new_ctx], 0).astype(F32)



# revision 4
# speedup vs baseline: 36.1163x; 36.1163x over previous
"""Trainium2 Bass kernel for nn_MessageGNN (gnn_message_passing).

Sharding: destination-sharded edges across 8 cores.  Core k owns clauses
[k*50000,(k+1)*50000) and vars [k*12500,(k+1)*12500) plus every edge whose
destination falls in its slice, so segment sums are fully core-local.
All 8 cores run ONE identical Bass program (SPMD); per-core variation
lives entirely in the data.  The edge schedule is made uniform by padding
every 512-destination window's edge count to the max over the 8 cores
(pad slots carry zero sat/x and weight 0, dst sentinel 70000).

The src-embedding gather is done host-side: each core's edge payload
ships as a pre-gathered feature-major x^T [128, S] f16 array, so the
device only streams contiguous DMA (no dma_gather).  Per 128-edge tile:
two PSUM-accumulated matmuls (x^T chunk + sat/bias chunk) produce
z[e,128]; a grouped Prelu + per-edge 1/deg scale gives m[e,128]; a
DVE-built one-hot [e,512] and one N=512 matmul accumulate the window's
h^T [128d, 512dst] in PSUM.  The node MLP for the same 512-node window
is 4 more matmuls (feats+bias / h / host-projected ctx one-hot / emb),
all weights f16; outputs leave d-major [128, Np] and the host
transposes.  Phase 3 (context update) runs on host from the returned
node embeddings.
"""

import sys

sys.path.insert(0, "/opt/trn_rl_repo")

import numpy as np

M = 8
WIN = 512
P = 128
GRP = 4
PAD_DST = 70000.0

F16 = np.float16
F32 = np.float32


def _side_prep(src, dst, sat, emb16, n_dst):
    """Sort edges by destination, build the shared padded window schedule
    and the per-core slot arrays (stacked core-major for shard_map)."""
    ndc = n_dst // M
    nwin = -(-ndc // WIN)
    order = np.argsort(dst, kind="stable")
    src_s, dst_s, sat_s = src[order], dst[order], sat[order]
    bounds = np.searchsorted(dst_s, np.arange(M + 1) * ndc)

    cnts = np.zeros((M, nwin), np.int64)
    wlocs = []
    for k in range(M):
        lo, hi = bounds[k], bounds[k + 1]
        wloc = (dst_s[lo:hi] - k * ndc) // WIN
        cnts[k] = np.bincount(wloc, minlength=nwin)
        wlocs.append(wloc)
    npad = np.maximum(((cnts.max(0) + P - 1) // P) * P, P)  # [nwin]
    offs = np.zeros(nwin + 1, np.int64)
    offs[1:] = np.cumsum(npad)
    S = int(offs[-1])
    T = S // P

    wrec = (1.0 / np.maximum(np.bincount(dst, minlength=n_dst), 1.0)).astype(F32)

    xT = np.zeros((M * P, S), F16)
    satT = np.zeros((M * 5, S), F16)
    dstw = np.full((M, S), PAD_DST, F32)
    wsc = np.zeros((M, S), F32)
    for k in range(M):
        lo, hi = bounds[k], bounds[k + 1]
        wloc = wlocs[k]
        runstart = np.zeros(nwin, np.int64)
        runstart[1:] = np.cumsum(cnts[k][:-1])
        n = hi - lo
        pos = offs[wloc] + (np.arange(n) - runstart[wloc])
        xT[k * P:(k + 1) * P, pos] = emb16[src_s[lo:hi]].T
        satT[k * 5:k * 5 + 4, pos] = sat_s[lo:hi].T.astype(F16)
        satT[k * 5 + 4, pos] = 1.0
        dstw[k, pos] = (dst_s[lo:hi] - k * ndc - wloc * WIN).astype(F32)
        wsc[k, pos] = wrec[dst_s[lo:hi]]
    dstT = np.ascontiguousarray(
        dstw.reshape(M, T, P).transpose(0, 2, 1).reshape(M * P, T))
    wT = np.ascontiguousarray(
        wsc.reshape(M, T, P).transpose(0, 2, 1).reshape(M * P, T))
    return dict(nwin=nwin, npad=npad.tolist(), S=S, T=T,
                xT=xT, satT=satT, dstT=dstT, wT=wT)


def _node_prep(feats, emb16, ctx_ids, n_nodes, Np):
    """Per-core node-phase arrays, stacked core-major."""
    nn = n_nodes // M
    nf = feats.shape[1]
    featsT = np.zeros((M * (nf + 1), Np), F16)
    embT = np.zeros((M * P, Np), F16)
    ohuT = np.zeros((M * 64, Np), F16)
    for k in range(M):
        fs, es, cs_ = (a[k * nn:(k + 1) * nn] for a in (feats, emb16, ctx_ids))
        featsT[k * (nf + 1):k * (nf + 1) + nf, :nn] = fs.T.astype(F16)
        featsT[k * (nf + 1) + nf, :nn] = 1.0
        embT[k * P:(k + 1) * P, :nn] = es.T
        ohuT[k * 64 + cs_, np.arange(nn)] = 1.0
    return featsT, embT, ohuT


def _build_program(meta):
    import concourse.mybir as mybir
    import concourse.tile as tile
    from concourse import bacc

    f16, f32 = mybir.dt.float16, mybir.dt.float32

    nc = bacc.Bacc("TRN2", target_bir_lowering=False, debug=False, num_devices=1)
    io = {}

    def dram(name, shape, dt, kind="ExternalInput"):
        io[name] = nc.dram_tensor(name, list(shape), dt, kind=kind)
        return io[name]

    for side in ("A", "B"):
        pl = meta[side]
        dram(f"xT{side}", [P, pl["S"]], f16)
        dram(f"satT{side}", [5, pl["S"]], f16)
        dram(f"dstT{side}", [P, pl["T"]], f32)
        dram(f"wT{side}", [P, pl["T"]], f32)
        dram(f"Wemb{side}", [P, P], f16)
        dram(f"Wsat{side}", [5, P], f16)
    for sd in ("C", "V"):
        Np = meta[f"Np{sd}"]
        dram(f"featsT{sd}", [17, Np], f16)
        dram(f"embT{sd}", [P, Np], f16)
        dram(f"ohuT{sd}", [64, Np], f16)
        dram(f"Wf{sd}", [17, P], f16)
        dram(f"Wh{sd}", [P, P], f16)
        dram(f"We{sd}", [P, P], f16)
        dram(f"ctxproj{sd}", [64, P], f16)
        dram(f"out{sd}", [P, Np], f16, kind="ExternalOutput")

    maxslot = max(max(meta["A"]["npad"]), max(meta["B"]["npad"]))

    with tile.TileContext(nc) as tc:
        with tc.tile_pool(name="const", bufs=1) as cpool, \
             tc.tile_pool(name="xs", bufs=3) as xpool, \
             tc.tile_pool(name="work", bufs=3) as wpool, \
             tc.tile_pool(name="psA", bufs=2, space="PSUM") as psA, \
             tc.tile_pool(name="psH", bufs=2, space="PSUM") as psH, \
             tc.tile_pool(name="psN", bufs=2, space="PSUM") as psN:

            iota_i = cpool.tile([P, WIN], mybir.dt.int32)
            nc.gpsimd.iota(iota_i[:], pattern=[[1, WIN]], base=0,
                           channel_multiplier=0)
            iota16 = cpool.tile([P, WIN], f16)
            nc.vector.tensor_copy(iota16[:], iota_i[:])

            wt = {}
            for nm in ("WembA", "WsatA", "WembB", "WsatB",
                       "WfC", "WhC", "WeC", "ctxprojC",
                       "WfV", "WhV", "WeV", "ctxprojV"):
                t = cpool.tile(list(io[nm].shape), f16, tag=nm)
                nc.sync.dma_start(t[:], io[nm][:])
                wt[nm] = t

            for side, sd in (("A", "C"), ("B", "V")):
                pl = meta[side]
                dstall = cpool.tile([P, pl["T"]], f32, tag=f"dst{side}")
                nc.sync.dma_start(dstall[:], io[f"dstT{side}"][:])
                wall = cpool.tile([P, pl["T"]], f32, tag=f"w{side}")
                nc.sync.dma_start(wall[:], io[f"wT{side}"][:])

                off = 0
                tbase = 0
                for w in range(pl["nwin"]):
                    slots = pl["npad"][w]
                    nt = slots // P
                    xw = xpool.tile([P, maxslot], f16, tag="xw")
                    nc.sync.dma_start(xw[:, :slots],
                                      io[f"xT{side}"][:, off:off + slots])
                    satw = xpool.tile([5, maxslot], f16, tag="satw")
                    nc.sync.dma_start(satw[:, :slots],
                                      io[f"satT{side}"][:, off:off + slots])
                    hps = psH.tile([P, WIN], f32, tag="hps")
                    mps = msb = None
                    for j in range(nt):
                        t = tbase + j
                        jj = j % GRP
                        if jj == 0:
                            gn = min(GRP, nt - j)
                            mps = psA.tile([P, GRP * P], f32, tag="mps")
                            msb = wpool.tile([P, GRP * P], f16, tag="msb")
                        nc.tensor.matmul(mps[:, jj * P:(jj + 1) * P],
                                         lhsT=xw[:, j * P:(j + 1) * P],
                                         rhs=wt[f"Wemb{side}"][:],
                                         start=True, stop=False)
                        nc.tensor.matmul(mps[:, jj * P:(jj + 1) * P],
                                         lhsT=satw[:, j * P:(j + 1) * P],
                                         rhs=wt[f"Wsat{side}"][:],
                                         start=False, stop=True)
                        # Prelu(z * w) == w * Prelu(z) for w >= 0: the 1/deg
                        # weight rides the activation's per-partition scale
                        nc.scalar.activation(
                            msb[:, jj * P:(jj + 1) * P],
                            mps[:, jj * P:(jj + 1) * P],
                            mybir.ActivationFunctionType.Prelu, alpha=0.1,
                            scale=wall[:, t:t + 1])
                        ohw = wpool.tile([P, WIN], f16, tag="ohw")
                        nc.vector.tensor_single_scalar(
                            out=ohw[:], in_=iota16[:],
                            scalar=dstall[:, t:t + 1],
                            op=mybir.AluOpType.is_equal)
                        nc.tensor.matmul(hps[:],
                                         lhsT=msb[:, jj * P:(jj + 1) * P],
                                         rhs=ohw[:],
                                         start=(j == 0), stop=(j == nt - 1),
                                         skip_group_check=True)
                    off += slots
                    tbase += nt

                    hT = wpool.tile([P, WIN], f16, tag="hT")
                    nc.vector.tensor_copy(hT[:], hps[:])

                    # node MLP for this 512-node window
                    cga = w * WIN
                    featsl = wpool.tile([17, WIN], f16, tag="featsl")
                    nc.sync.dma_start(featsl[:], io[f"featsT{sd}"][:, cga:cga + WIN])
                    embl = wpool.tile([P, WIN], f16, tag="embl")
                    nc.sync.dma_start(embl[:], io[f"embT{sd}"][:, cga:cga + WIN])
                    ohul = wpool.tile([64, WIN], f16, tag="ohul")
                    nc.sync.dma_start(ohul[:], io[f"ohuT{sd}"][:, cga:cga + WIN])
                    nps = psN.tile([P, WIN], f32, tag="nps")
                    nc.tensor.matmul(nps[:], lhsT=wt[f"Wf{sd}"][:], rhs=featsl[:],
                                     start=True, stop=False)
                    nc.tensor.matmul(nps[:], lhsT=wt[f"Wh{sd}"][:], rhs=hT[:],
                                     start=False, stop=False)
                    nc.tensor.matmul(nps[:], lhsT=wt[f"ctxproj{sd}"][:], rhs=ohul[:],
                                     start=False, stop=False)
                    nc.tensor.matmul(nps[:], lhsT=wt[f"We{sd}"][:], rhs=embl[:],
                                     start=False, stop=True)
                    nsb = wpool.tile([P, WIN], f16, tag="nsb")
                    nc.scalar.activation(nsb[:], nps[:],
                                         mybir.ActivationFunctionType.Prelu,
                                         alpha=0.1)
                    nc.sync.dma_start(io[f"out{sd}"][:, cga:cga + WIN], nsb[:])
    nc.compile()
    return nc


_spmd_state = {}

REPLICATED = ("WembA", "WsatA", "WembB", "WsatB",
              "WfC", "WhC", "WeC", "ctxprojC",
              "WfV", "WhV", "WeV", "ctxprojV")


def _run_spmd(nc, stacked_map, repl_map):
    """One shard_map dispatch running the identical program on all 8 cores.

    stacked_map[nm] is the core-major stacked array [M*rows, ...]; repl_map
    holds the small replicated weights."""
    import concourse.mybir as mybir
    import jax
    from concourse.bass2jax import (_bass_exec_p, install_neuronx_cc_hook,
                                    partition_id_tensor)
    from jax.experimental.shard_map import shard_map
    from jax.sharding import Mesh, NamedSharding, PartitionSpec

    install_neuronx_cc_hook()
    partition_name = nc.partition_id_tensor.name if nc.partition_id_tensor else None
    in_names, out_names, out_avals, zero_shapes = [], [], [], []
    for alloc in nc.m.functions[0].allocations:
        if not isinstance(alloc, mybir.MemoryLocationSet):
            continue
        name = alloc.memorylocations[0].name
        if alloc.kind == "ExternalInput":
            if name != partition_name:
                in_names.append(name)
        elif alloc.kind == "ExternalOutput":
            shape = tuple(alloc.tensor_shape)
            dtype = mybir.dt.np(alloc.dtype)
            out_names.append(name)
            out_avals.append(jax.core.ShapedArray(shape, dtype))
            zero_shapes.append((shape, dtype))
    n_params = len(in_names)
    n_outs = len(out_names)
    all_names = list(in_names) + list(out_names)
    if partition_name is not None:
        all_names.append(partition_name)
    donate = tuple(range(n_params, n_params + n_outs))

    def _body(*args):
        operands = list(args)
        if partition_name is not None:
            operands.append(partition_id_tensor())
        return tuple(_bass_exec_p.bind(
            *operands, out_avals=tuple(out_avals), in_names=tuple(all_names),
            out_names=tuple(out_names), lowering_input_output_aliases=(),
            sim_require_finite=True, sim_require_nnan=True, nc=nc))

    devices = jax.devices()[:M]
    mesh = Mesh(np.asarray(devices), ("core",))
    in_specs = tuple(
        PartitionSpec() if nm in REPLICATED else PartitionSpec("core")
        for nm in in_names) + (PartitionSpec("core"),) * n_outs
    out_specs = (PartitionSpec("core"),) * n_outs
    sharded = jax.jit(
        shard_map(_body, mesh=mesh, in_specs=in_specs, out_specs=out_specs,
                  check_rep=False),
        donate_argnums=donate, keep_unused=True)

    sh_core = NamedSharding(mesh, PartitionSpec("core"))
    sh_repl = NamedSharding(mesh, PartitionSpec())

    d_ins = []
    for nm in in_names:
        if nm in REPLICATED:
            d_ins.append(jax.device_put(repl_map[nm], sh_repl))
        else:
            d_ins.append(jax.device_put(stacked_map[nm], sh_core))
    d_zeros = [jax.device_put(np.zeros((M * s[0], *s[1:]), dt), sh_core)
               for (s, dt) in zero_shapes]

    outs = sharded(*d_ins, *d_zeros)
    host = {nm: np.asarray(o) for nm, o in zip(out_names, outs)}

    _spmd_state.clear()
    _spmd_state.update(fn=sharded, d_ins=d_ins, outs=list(outs),
                       out_names=out_names, sh_core=sh_core,
                       zero_shapes=zero_shapes, nc=nc)
    return host


def _redispatch():
    """Re-run the compiled program once (fresh zero output buffers)."""
    import jax
    st = _spmd_state
    d_zeros = [jax.device_put(np.zeros((M * s[0], *s[1:]), dt), st["sh_core"])
               for (s, dt) in st["zero_shapes"]]
    outs = st["fn"](*st["d_ins"], *d_zeros)
    st["outs"] = list(outs)
    return {nm: np.asarray(o) for nm, o in zip(st["out_names"], outs)}


def _assemble(host_out, n_nodes, Np):
    """[M*P, Np] core-major d-major output -> [n_nodes, 128] f32."""
    nn = n_nodes // M
    parts = [host_out[k * P:(k + 1) * P, :nn] for k in range(M)]
    return np.concatenate(parts, axis=1).T.astype(F32)


def _segmean(x, ids, n):
    order = np.argsort(ids, kind="stable")
    xs = x[order]
    ids_s = ids[order]
    starts = np.searchsorted(ids_s, np.arange(n))
    cnt = np.bincount(ids, minlength=n).astype(F32)
    sums = np.zeros((n, x.shape[1]), F32)
    nz = cnt > 0
    red = np.add.reduceat(xs, starts[nz], axis=0) if nz.any() else None
    if red is not None:
        sums[nz] = red
    return sums / np.maximum(cnt, 1.0)[:, None]


def _spot_check(new_clause, new_var, inp, a_src, a_dst, c_src, c_dst,
                clause_ctx, var_ctx, n_sample=48, seed=7):
    """Exact host recomputation of a few output nodes.  Clean device runs
    differ by ~3e-3 absmax (f16 path); silent corruption differs by ~2.5 —
    threshold 0.15 separates them cleanly."""
    rng = np.random.default_rng(seed)
    lrelu = lambda x: np.where(x >= 0, x, 0.1 * x)
    ok = True
    for (emb_src, src, dst, sat, Wm, bm, feats, Wn, bn, ctx_ids, emb_self,
         got_all) in (
            (inp["var_emb"], a_src, a_dst, inp["edge_sat_vc"], inp["W_vc"],
             inp["b_vc"], inp["clause_feats"], inp["W_c"], inp["b_c"],
             clause_ctx, inp["clause_emb"], new_clause),
            (inp["clause_emb"], c_src, c_dst, inp["edge_sat_cv"], inp["W_cv"],
             inp["b_cv"], inp["var_feats"], inp["W_v"], inp["b_v"],
             var_ctx, inp["var_emb"], new_var)):
        n_nodes = feats.shape[0]
        sample = rng.choice(n_nodes, size=min(n_sample, n_nodes), replace=False)
        sel = np.isin(dst, sample)
        es, ds = src[sel], dst[sel]
        m = lrelu(np.concatenate([sat[sel].astype(F32),
                                  emb_src[es].astype(F32)], 1)
                  @ Wm.astype(F32) + bm.astype(F32))
        h = np.zeros((n_nodes, 128), F32)
        np.add.at(h, ds, m)
        cnt = np.bincount(dst, minlength=n_nodes).astype(F32)
        h = h / np.maximum(cnt, 1.0)[:, None]
        ctx_e = inp["ctx_emb"][ctx_ids[sample]].astype(F32)
        z = np.concatenate([feats[sample].astype(F32), h[sample], ctx_e,
                            emb_self[sample].astype(F32)], 1) \
            @ Wn.astype(F32) + bn.astype(F32)
        ref = lrelu(z)
        if np.abs(got_all[sample] - ref).max() > 0.15:
            ok = False
    return ok


def kernel(**inputs):
    inp = {k: np.asarray(v) for k, v in inputs.items()}
    var_emb, clause_emb, ctx_emb = inp["var_emb"], inp["clause_emb"], inp["ctx_emb"]
    nv, ncl, nu = var_emb.shape[0], clause_emb.shape[0], ctx_emb.shape[0]
    cs, vs = ncl // M, nv // M

    a_src = inp["assigns_src"].astype(np.int64)
    a_dst = inp["assigns_dst"].astype(np.int64)
    c_src = inp["contains_src"].astype(np.int64)
    c_dst = inp["contains_dst"].astype(np.int64)
    var_ctx = inp["var_ctx"].astype(np.int64)
    clause_ctx = inp["clause_ctx"].astype(np.int64)

    emb16V = var_emb.astype(F16)
    emb16C = clause_emb.astype(F16)

    planA = _side_prep(a_src, a_dst, inp["edge_sat_vc"], emb16V, ncl)
    planB = _side_prep(c_src, c_dst, inp["edge_sat_cv"], emb16C, nv)

    NpC = planA["nwin"] * WIN
    NpV = planB["nwin"] * WIN
    featsTC, embTC, ohuTC = _node_prep(inp["clause_feats"], emb16C,
                                       clause_ctx, ncl, NpC)
    featsTV, embTV, ohuTV = _node_prep(inp["var_feats"], emb16V,
                                       var_ctx, nv, NpV)

    def node_w(Wn, bn):
        Wn, bn = Wn.astype(F32), bn.astype(F32)
        nf = Wn.shape[0] - 3 * 128
        Wf = np.vstack([Wn[:nf], bn[None, :]]).astype(F16)
        Wh = np.ascontiguousarray(Wn[nf:nf + 128]).astype(F16)
        ctxproj = (ctx_emb.astype(F32) @ Wn[nf + 128:nf + 256]).astype(F16)
        We = np.ascontiguousarray(Wn[nf + 256:nf + 384]).astype(F16)
        return Wf, Wh, ctxproj, We

    WfC, WhC, ctxprojC, WeC = node_w(inp["W_c"], inp["b_c"])
    WfV, WhV, ctxprojV, WeV = node_w(inp["W_v"], inp["b_v"])

    W_vc, b_vc = inp["W_vc"].astype(F32), inp["b_vc"].astype(F32)
    W_cv, b_cv = inp["W_cv"].astype(F32), inp["b_cv"].astype(F32)
    repl_map = dict(
        WembA=np.ascontiguousarray(W_vc[4:132]).astype(F16),
        WsatA=np.vstack([W_vc[:4], b_vc[None, :]]).astype(F16),
        WembB=np.ascontiguousarray(W_cv[4:132]).astype(F16),
        WsatB=np.vstack([W_cv[:4], b_cv[None, :]]).astype(F16),
        WfC=WfC, WhC=WhC, WeC=WeC, ctxprojC=ctxprojC,
        WfV=WfV, WhV=WhV, WeV=WeV, ctxprojV=ctxprojV,
    )
    stacked_map = dict(
        xTA=planA["xT"], satTA=planA["satT"], dstTA=planA["dstT"], wTA=planA["wT"],
        xTB=planB["xT"], satTB=planB["satT"], dstTB=planB["dstT"], wTB=planB["wT"],
        featsTC=featsTC, embTC=embTC, ohuTC=ohuTC,
        featsTV=featsTV, embTV=embTV, ohuTV=ohuTV,
    )

    meta = dict(
        A=dict(nwin=planA["nwin"], npad=planA["npad"], S=planA["S"], T=planA["T"]),
        B=dict(nwin=planB["nwin"], npad=planB["npad"], S=planB["S"], T=planB["T"]),
        NpC=NpC, NpV=NpV)
    nc = _build_program(meta)
    host = _run_spmd(nc, stacked_map, repl_map)

    new_clause = _assemble(host["outC"], ncl, NpC)
    new_var = _assemble(host["outV"], nv, NpV)
    # guard against rare silent corruption on the terminal
    for _ in range(2):
        if _spot_check(new_clause, new_var, inp, a_src, a_dst, c_src, c_dst,
                       clause_ctx, var_ctx):
            break
        host = _redispatch()
        new_clause = _assemble(host["outC"], ncl, NpC)
        new_var = _assemble(host["outV"], nv, NpV)

    # Phase 3 on host
    c_ctx = _segmean(new_clause, clause_ctx, nu)
    v_ctx = _segmean(new_var, var_ctx, nu)
    zu = np.concatenate([inp["ctx_feats"].astype(F32), c_ctx, v_ctx,
                         ctx_emb.astype(F32)], 1) @ inp["W_u"].astype(F32) \
        + inp["b_u"].astype(F32)
    new_ctx = np.where(zu >= 0, zu, 0.1 * zu).astype(F32)

    return np.concatenate([new_clause, new_var, new_ctx], 0).astype(F32)


# revision 9
# speedup vs baseline: 75.5462x; 2.0917x over previous
"""Trainium2 Bass kernel for nn_MessageGNN (gnn_message_passing).

Sharding: destination-sharded edges across 8 cores.  Core k owns clauses
[k*50000,(k+1)*50000) and vars [k*12500,(k+1)*12500) plus every edge whose
destination falls in its slice, so segment sums are fully core-local.
All 8 cores run ONE identical Bass program (SPMD); per-core variation
lives entirely in the data.  The edge schedule is made uniform by padding
every 256-destination window's edge count to the max over the 8 cores
(pad slots carry zero x, weight 0, dst sentinel 70000).

Host-side preprocessing folds everything per-edge into one pre-gathered
feature-major payload: x'_e = x_src(e) + ([sat_e, 1] @ [W_sat; b] @
W_emb^{-1}), so the edge MLP is a single f16 matmul z = x' W_emb per
128-edge tile (W_emb is square and well-conditioned; the fold costs
~6e-3 absolute at the edge stage, well under the 2e-2 gate).  A DVE
one-hot (iota==dst)*1/deg [e,256] and one N=256 matmul accumulate each
window's h^T [128d, 256dst] in PSUM; Prelu runs grouped on the Scalar
engine.  The node MLP consumes two windows at a time (N=512): 4 matmuls
(feats+bias / h / host-projected ctx one-hot / emb), outputs leave
d-major [128, Np] and the host transposes.  Phase 3 (context update)
runs on host from the returned node embeddings.
"""

import sys

sys.path.insert(0, "/opt/trn_rl_repo")

import numpy as np

M = 8
WIN = 256
P = 128
GRP = 4
PAD_DST = 70000.0

F16 = np.float16
F32 = np.float32


def _side_prep(src, dst, sat, emb32, WsbInv, n_dst):
    """Sort edges by destination, build the shared padded window schedule
    and the per-core slot arrays (stacked core-major for shard_map).

    x' = emb[src] + [sat, 1] @ WsbInv  (sat/bias folded into emb space)."""
    ndc = n_dst // M
    nwin = -(-ndc // WIN)
    order = np.argsort(dst, kind="stable")
    src_s, dst_s, sat_s = src[order], dst[order], sat[order]
    bounds = np.searchsorted(dst_s, np.arange(M + 1) * ndc)

    cnts = np.zeros((M, nwin), np.int64)
    wlocs = []
    for k in range(M):
        lo, hi = bounds[k], bounds[k + 1]
        wloc = (dst_s[lo:hi] - k * ndc) // WIN
        cnts[k] = np.bincount(wloc, minlength=nwin)
        wlocs.append(wloc)
    npad = np.maximum(((cnts.max(0) + P - 1) // P) * P, P)  # [nwin]
    offs = np.zeros(nwin + 1, np.int64)
    offs[1:] = np.cumsum(npad)
    S = int(offs[-1])
    T = S // P

    wrec = (1.0 / np.maximum(np.bincount(dst, minlength=n_dst), 1.0)).astype(F32)

    xT = np.zeros((M * P, S), F16)
    dstw = np.full((M, S), PAD_DST, F32)
    wsc = np.zeros((M, S), F32)
    for k in range(M):
        lo, hi = bounds[k], bounds[k + 1]
        wloc = wlocs[k]
        runstart = np.zeros(nwin, np.int64)
        runstart[1:] = np.cumsum(cnts[k][:-1])
        n = hi - lo
        pos = offs[wloc] + (np.arange(n) - runstart[wloc])
        xp = emb32[src_s[lo:hi]] + sat_s[lo:hi].astype(F32) @ WsbInv[:4] \
            + WsbInv[4]
        xT[k * P:(k + 1) * P, pos] = xp.astype(F16).T
        dstw[k, pos] = (dst_s[lo:hi] - k * ndc - wloc * WIN).astype(F32)
        wsc[k, pos] = wrec[dst_s[lo:hi]]
    dstT = np.ascontiguousarray(
        dstw.reshape(M, T, P).transpose(0, 2, 1).reshape(M * P, T))
    wT = np.ascontiguousarray(
        wsc.reshape(M, T, P).transpose(0, 2, 1).reshape(M * P, T))
    return dict(nwin=nwin, npad=npad.tolist(), S=S, T=T,
                xT=xT, dstT=dstT, wT=wT)


def _node_prep(feats, emb16, ctx_ids, n_nodes, Np):
    """Per-core node-phase arrays, stacked core-major."""
    nn = n_nodes // M
    nf = feats.shape[1]
    featsT = np.zeros((M * (nf + 1), Np), F16)
    embT = np.zeros((M * P, Np), F16)
    ohuT = np.zeros((M * 64, Np), F16)
    for k in range(M):
        fs, es, cs_ = (a[k * nn:(k + 1) * nn] for a in (feats, emb16, ctx_ids))
        featsT[k * (nf + 1):k * (nf + 1) + nf, :nn] = fs.T.astype(F16)
        featsT[k * (nf + 1) + nf, :nn] = 1.0
        embT[k * P:(k + 1) * P, :nn] = es.T
        ohuT[k * 64 + cs_, np.arange(nn)] = 1.0
    return featsT, embT, ohuT


def _build_program(meta):
    import concourse.mybir as mybir
    import concourse.tile as tile
    from concourse import bacc

    f16, f32 = mybir.dt.float16, mybir.dt.float32

    nc = bacc.Bacc("TRN2", target_bir_lowering=False, debug=False, num_devices=1)
    io = {}

    def dram(name, shape, dt, kind="ExternalInput"):
        io[name] = nc.dram_tensor(name, list(shape), dt, kind=kind)
        return io[name]

    for side in ("A", "B"):
        pl = meta[side]
        dram(f"xT{side}", [P, pl["S"]], f16)
        dram(f"dstT{side}", [P, pl["T"]], f32)
        dram(f"wT{side}", [P, pl["T"]], f32)
        dram(f"Wemb{side}", [P, P], f16)
    for sd in ("C", "V"):
        Np = meta[f"Np{sd}"]
        dram(f"featsT{sd}", [17, Np], f16)
        dram(f"embT{sd}", [P, Np], f16)
        dram(f"ohuT{sd}", [64, Np], f16)
        dram(f"Wf{sd}", [17, P], f16)
        dram(f"Wh{sd}", [P, P], f16)
        dram(f"We{sd}", [P, P], f16)
        dram(f"ctxproj{sd}", [64, P], f16)
        dram(f"out{sd}", [P, Np], f16, kind="ExternalOutput")

    maxslot = max(max(meta["A"]["npad"]), max(meta["B"]["npad"]))

    with tile.TileContext(nc) as tc:
        with tc.tile_pool(name="const", bufs=1) as cpool, \
             tc.tile_pool(name="xs", bufs=3) as xpool, \
             tc.tile_pool(name="work", bufs=4) as wpool, \
             tc.tile_pool(name="oh", bufs=8) as opool, \
             tc.tile_pool(name="psA", bufs=2, space="PSUM") as psA, \
             tc.tile_pool(name="psH", bufs=2, space="PSUM") as psH, \
             tc.tile_pool(name="psN", bufs=2, space="PSUM") as psN:

            iota_i = cpool.tile([P, WIN], mybir.dt.int32)
            nc.gpsimd.iota(iota_i[:], pattern=[[1, WIN]], base=0,
                           channel_multiplier=0)
            iota16 = cpool.tile([P, WIN], f16)
            nc.vector.tensor_copy(iota16[:], iota_i[:])

            wt = {}
            for nm in ("WembA", "WembB",
                       "WfC", "WhC", "WeC", "ctxprojC",
                       "WfV", "WhV", "WeV", "ctxprojV"):
                t = cpool.tile(list(io[nm].shape), f16, tag=nm)
                nc.sync.dma_start(t[:], io[nm][:])
                wt[nm] = t

            for side, sd in (("A", "C"), ("B", "V")):
                pl = meta[side]
                nwin = pl["nwin"]
                dstall = cpool.tile([P, pl["T"]], f32, tag=f"dst{side}")
                nc.sync.dma_start(dstall[:], io[f"dstT{side}"][:])
                wall = cpool.tile([P, pl["T"]], f32, tag=f"w{side}")
                nc.sync.dma_start(wall[:], io[f"wT{side}"][:])

                off = 0
                tbase = 0
                hTpair = None
                for w in range(nwin):
                    slots = pl["npad"][w]
                    nt = slots // P
                    xw = xpool.tile([P, maxslot], f16, tag="xw")
                    nc.sync.dma_start(xw[:, :slots],
                                      io[f"xT{side}"][:, off:off + slots])
                    hps = psH.tile([P, WIN], f32, tag="hps")
                    for j0 in range(0, nt, GRP):
                        gn = min(GRP, nt - j0)
                        mps = psA.tile([P, GRP * P], f32, tag="mps")
                        msb = wpool.tile([P, GRP * P], f16, tag="msb")
                        ohws = []
                        for jj in range(gn):
                            t = tbase + j0 + jj
                            nc.tensor.matmul(mps[:, jj * P:(jj + 1) * P],
                                             lhsT=xw[:, (j0 + jj) * P:
                                                     (j0 + jj + 1) * P],
                                             rhs=wt[f"Wemb{side}"][:],
                                             start=True, stop=True)
                            # one-hot of dst scaled by 1/deg (0 on pad slots)
                            ohw = opool.tile([P, WIN], f16, tag="ohw")
                            nc.vector.scalar_tensor_tensor(
                                out=ohw[:], in0=iota16[:],
                                scalar=dstall[:, t:t + 1],
                                in1=wall[:, t:t + 1].to_broadcast([P, WIN]),
                                op0=mybir.AluOpType.is_equal,
                                op1=mybir.AluOpType.mult)
                            ohws.append(ohw)
                        nc.scalar.activation(
                            msb[:, :gn * P], mps[:, :gn * P],
                            mybir.ActivationFunctionType.Prelu, alpha=0.1)
                        for jj in range(gn):
                            j = j0 + jj
                            nc.tensor.matmul(hps[:],
                                             lhsT=msb[:, jj * P:(jj + 1) * P],
                                             rhs=ohws[jj][:],
                                             start=(j == 0), stop=(j == nt - 1),
                                             skip_group_check=True)
                    off += slots
                    tbase += nt

                    if hTpair is None:
                        hTpair = wpool.tile([P, 2 * WIN], f16, tag="hT")
                    nc.vector.tensor_copy(
                        hTpair[:, (w % 2) * WIN:(w % 2) * WIN + WIN], hps[:])

                    if w % 2 == 0 and w != nwin - 1:
                        continue
                    # node MLP for the last 1-2 windows (N = 512 or 256)
                    nw = WIN if w % 2 == 0 else 2 * WIN
                    cga = (w + 1) * WIN - nw
                    featsl = wpool.tile([17, 2 * WIN], f16, tag="featsl")
                    nc.sync.dma_start(featsl[:, :nw],
                                      io[f"featsT{sd}"][:, cga:cga + nw])
                    embl = wpool.tile([P, 2 * WIN], f16, tag="embl")
                    nc.sync.dma_start(embl[:, :nw],
                                      io[f"embT{sd}"][:, cga:cga + nw])
                    ohul = wpool.tile([64, 2 * WIN], f16, tag="ohul")
                    nc.sync.dma_start(ohul[:, :nw],
                                      io[f"ohuT{sd}"][:, cga:cga + nw])
                    nps = psN.tile([P, 2 * WIN], f32, tag="nps")
                    nc.tensor.matmul(nps[:, :nw], lhsT=wt[f"Wf{sd}"][:],
                                     rhs=featsl[:, :nw], start=True, stop=False)
                    nc.tensor.matmul(nps[:, :nw], lhsT=wt[f"Wh{sd}"][:],
                                     rhs=hTpair[:, :nw],
                                     start=False, stop=False)
                    nc.tensor.matmul(nps[:, :nw], lhsT=wt[f"ctxproj{sd}"][:],
                                     rhs=ohul[:, :nw], start=False, stop=False)
                    nc.tensor.matmul(nps[:, :nw], lhsT=wt[f"We{sd}"][:],
                                     rhs=embl[:, :nw], start=False, stop=True)
                    nsb = wpool.tile([P, 2 * WIN], f16, tag="nsb")
                    nc.scalar.activation(nsb[:, :nw], nps[:, :nw],
                                         mybir.ActivationFunctionType.Prelu,
                                         alpha=0.1)
                    nc.sync.dma_start(io[f"out{sd}"][:, cga:cga + nw],
                                      nsb[:, :nw])
                    hTpair = None
    nc.compile()
    return nc


_spmd_state = {}

REPLICATED = ("WembA", "WembB",
              "WfC", "WhC", "WeC", "ctxprojC",
              "WfV", "WhV", "WeV", "ctxprojV")


def _run_spmd(nc, stacked_map, repl_map):
    """One shard_map dispatch running the identical program on all 8 cores.

    stacked_map[nm] is the core-major stacked array [M*rows, ...]; repl_map
    holds the small replicated weights."""
    import concourse.mybir as mybir
    import jax
    from concourse.bass2jax import (_bass_exec_p, install_neuronx_cc_hook,
                                    partition_id_tensor)
    from jax.experimental.shard_map import shard_map
    from jax.sharding import Mesh, NamedSharding, PartitionSpec

    install_neuronx_cc_hook()
    partition_name = nc.partition_id_tensor.name if nc.partition_id_tensor else None
    in_names, out_names, out_avals, zero_shapes = [], [], [], []
    for alloc in nc.m.functions[0].allocations:
        if not isinstance(alloc, mybir.MemoryLocationSet):
            continue
        name = alloc.memorylocations[0].name
        if alloc.kind == "ExternalInput":
            if name != partition_name:
                in_names.append(name)
        elif alloc.kind == "ExternalOutput":
            shape = tuple(alloc.tensor_shape)
            dtype = mybir.dt.np(alloc.dtype)
            out_names.append(name)
            out_avals.append(jax.core.ShapedArray(shape, dtype))
            zero_shapes.append((shape, dtype))
    n_params = len(in_names)
    n_outs = len(out_names)
    all_names = list(in_names) + list(out_names)
    if partition_name is not None:
        all_names.append(partition_name)
    donate = tuple(range(n_params, n_params + n_outs))

    def _body(*args):
        operands = list(args)
        if partition_name is not None:
            operands.append(partition_id_tensor())
        return tuple(_bass_exec_p.bind(
            *operands, out_avals=tuple(out_avals), in_names=tuple(all_names),
            out_names=tuple(out_names), lowering_input_output_aliases=(),
            sim_require_finite=True, sim_require_nnan=True, nc=nc))

    devices = jax.devices()[:M]
    mesh = Mesh(np.asarray(devices), ("core",))
    in_specs = tuple(
        PartitionSpec() if nm in REPLICATED else PartitionSpec("core")
        for nm in in_names) + (PartitionSpec("core"),) * n_outs
    out_specs = (PartitionSpec("core"),) * n_outs
    sharded = jax.jit(
        shard_map(_body, mesh=mesh, in_specs=in_specs, out_specs=out_specs,
                  check_rep=False),
        donate_argnums=donate, keep_unused=True)

    sh_core = NamedSharding(mesh, PartitionSpec("core"))
    sh_repl = NamedSharding(mesh, PartitionSpec())

    d_ins = []
    for nm in in_names:
        if nm in REPLICATED:
            d_ins.append(jax.device_put(repl_map[nm], sh_repl))
        else:
            d_ins.append(jax.device_put(stacked_map[nm], sh_core))
    d_zeros = [jax.device_put(np.zeros((M * s[0], *s[1:]), dt), sh_core)
               for (s, dt) in zero_shapes]

    outs = sharded(*d_ins, *d_zeros)
    host = {nm: np.asarray(o) for nm, o in zip(out_names, outs)}

    _spmd_state.clear()
    _spmd_state.update(fn=sharded, d_ins=d_ins, outs=list(outs),
                       out_names=out_names, sh_core=sh_core,
                       zero_shapes=zero_shapes, nc=nc)
    return host


def _redispatch():
    """Re-run the compiled program once (fresh zero output buffers)."""
    import jax
    st = _spmd_state
    d_zeros = [jax.device_put(np.zeros((M * s[0], *s[1:]), dt), st["sh_core"])
               for (s, dt) in st["zero_shapes"]]
    outs = st["fn"](*st["d_ins"], *d_zeros)
    st["outs"] = list(outs)
    return {nm: np.asarray(o) for nm, o in zip(st["out_names"], outs)}


def _assemble(host_out, n_nodes, Np):
    """[M*P, Np] core-major d-major output -> [n_nodes, 128] f32."""
    nn = n_nodes // M
    parts = [host_out[k * P:(k + 1) * P, :nn] for k in range(M)]
    return np.concatenate(parts, axis=1).T.astype(F32)


def _segmean(x, ids, n):
    order = np.argsort(ids, kind="stable")
    xs = x[order]
    ids_s = ids[order]
    starts = np.searchsorted(ids_s, np.arange(n))
    cnt = np.bincount(ids, minlength=n).astype(F32)
    sums = np.zeros((n, x.shape[1]), F32)
    nz = cnt > 0
    if nz.any():
        sums[nz] = np.add.reduceat(xs, starts[nz], axis=0)
    return sums / np.maximum(cnt, 1.0)[:, None]


def _spot_check(new_clause, new_var, inp, a_src, a_dst, c_src, c_dst,
                clause_ctx, var_ctx, n_sample=48, seed=7):
    """Exact host recomputation of a few output nodes.  Clean device runs
    differ by <2e-2 absmax (f16 + sat-fold path); silent corruption differs
    by ~2.5 — threshold 0.2 separates them cleanly."""
    rng = np.random.default_rng(seed)
    lrelu = lambda x: np.where(x >= 0, x, 0.1 * x)
    ok = True
    for (emb_src, src, dst, sat, Wm, bm, feats, Wn, bn, ctx_ids, emb_self,
         got_all) in (
            (inp["var_emb"], a_src, a_dst, inp["edge_sat_vc"], inp["W_vc"],
             inp["b_vc"], inp["clause_feats"], inp["W_c"], inp["b_c"],
             clause_ctx, inp["clause_emb"], new_clause),
            (inp["clause_emb"], c_src, c_dst, inp["edge_sat_cv"], inp["W_cv"],
             inp["b_cv"], inp["var_feats"], inp["W_v"], inp["b_v"],
             var_ctx, inp["var_emb"], new_var)):
        n_nodes = feats.shape[0]
        sample = rng.choice(n_nodes, size=min(n_sample, n_nodes), replace=False)
        sel = np.isin(dst, sample)
        es, ds = src[sel], dst[sel]
        m = lrelu(np.concatenate([sat[sel].astype(F32),
                                  emb_src[es].astype(F32)], 1)
                  @ Wm.astype(F32) + bm.astype(F32))
        h = np.zeros((n_nodes, 128), F32)
        np.add.at(h, ds, m)
        cnt = np.bincount(dst, minlength=n_nodes).astype(F32)
        h = h / np.maximum(cnt, 1.0)[:, None]
        ctx_e = inp["ctx_emb"][ctx_ids[sample]].astype(F32)
        z = np.concatenate([feats[sample].astype(F32), h[sample], ctx_e,
                            emb_self[sample].astype(F32)], 1) \
            @ Wn.astype(F32) + bn.astype(F32)
        ref = lrelu(z)
        if np.abs(got_all[sample] - ref).max() > 0.2:
            ok = False
    return ok


def kernel(**inputs):
    inp = {k: np.asarray(v) for k, v in inputs.items()}
    var_emb, clause_emb, ctx_emb = inp["var_emb"], inp["clause_emb"], inp["ctx_emb"]
    nv, ncl, nu = var_emb.shape[0], clause_emb.shape[0], ctx_emb.shape[0]

    a_src = inp["assigns_src"].astype(np.int64)
    a_dst = inp["assigns_dst"].astype(np.int64)
    c_src = inp["contains_src"].astype(np.int64)
    c_dst = inp["contains_dst"].astype(np.int64)
    var_ctx = inp["var_ctx"].astype(np.int64)
    clause_ctx = inp["clause_ctx"].astype(np.int64)

    # fold [sat, 1] @ [W_sat; b] through W_emb^{-1} into the gathered x rows
    W_vc, b_vc = inp["W_vc"].astype(F32), inp["b_vc"].astype(F32)
    W_cv, b_cv = inp["W_cv"].astype(F32), inp["b_cv"].astype(F32)
    WsbInvA = (np.vstack([W_vc[:4], b_vc[None, :]]).astype(np.float64)
               @ np.linalg.inv(W_vc[4:132].astype(np.float64))).astype(F32)
    WsbInvB = (np.vstack([W_cv[:4], b_cv[None, :]]).astype(np.float64)
               @ np.linalg.inv(W_cv[4:132].astype(np.float64))).astype(F32)

    planA = _side_prep(a_src, a_dst, inp["edge_sat_vc"],
                       var_emb.astype(F32), WsbInvA, ncl)
    planB = _side_prep(c_src, c_dst, inp["edge_sat_cv"],
                       clause_emb.astype(F32), WsbInvB, nv)

    NpC = planA["nwin"] * WIN
    NpV = planB["nwin"] * WIN
    emb16V = var_emb.astype(F16)
    emb16C = clause_emb.astype(F16)
    featsTC, embTC, ohuTC = _node_prep(inp["clause_feats"], emb16C,
                                       clause_ctx, ncl, NpC)
    featsTV, embTV, ohuTV = _node_prep(inp["var_feats"], emb16V,
                                       var_ctx, nv, NpV)

    def node_w(Wn, bn):
        Wn, bn = Wn.astype(F32), bn.astype(F32)
        nf = Wn.shape[0] - 3 * 128
        Wf = np.vstack([Wn[:nf], bn[None, :]]).astype(F16)
        Wh = np.ascontiguousarray(Wn[nf:nf + 128]).astype(F16)
        ctxproj = (ctx_emb.astype(F32) @ Wn[nf + 128:nf + 256]).astype(F16)
        We = np.ascontiguousarray(Wn[nf + 256:nf + 384]).astype(F16)
        return Wf, Wh, ctxproj, We

    WfC, WhC, ctxprojC, WeC = node_w(inp["W_c"], inp["b_c"])
    WfV, WhV, ctxprojV, WeV = node_w(inp["W_v"], inp["b_v"])

    repl_map = dict(
        WembA=np.ascontiguousarray(W_vc[4:132]).astype(F16),
        WembB=np.ascontiguousarray(W_cv[4:132]).astype(F16),
        WfC=WfC, WhC=WhC, WeC=WeC, ctxprojC=ctxprojC,
        WfV=WfV, WhV=WhV, WeV=WeV, ctxprojV=ctxprojV,
    )
    stacked_map = dict(
        xTA=planA["xT"], dstTA=planA["dstT"], wTA=planA["wT"],
        xTB=planB["xT"], dstTB=planB["dstT"], wTB=planB["wT"],
        featsTC=featsTC, embTC=embTC, ohuTC=ohuTC,
        featsTV=featsTV, embTV=embTV, ohuTV=ohuTV,
    )

    meta = dict(
        A=dict(nwin=planA["nwin"], npad=planA["npad"], S=planA["S"], T=planA["T"]),
        B=dict(nwin=planB["nwin"], npad=planB["npad"], S=planB["S"], T=planB["T"]),
        NpC=NpC, NpV=NpV)
    nc = _build_program(meta)
    host = _run_spmd(nc, stacked_map, repl_map)

    new_clause = _assemble(host["outC"], ncl, NpC)
    new_var = _assemble(host["outV"], nv, NpV)
    # guard against rare silent corruption on the terminal
    for _ in range(2):
        if _spot_check(new_clause, new_var, inp, a_src, a_dst, c_src, c_dst,
                       clause_ctx, var_ctx):
            break
        host = _redispatch()
        new_clause = _assemble(host["outC"], ncl, NpC)
        new_var = _assemble(host["outV"], nv, NpV)

    # Phase 3 on host
    c_ctx = _segmean(new_clause, clause_ctx, nu)
    v_ctx = _segmean(new_var, var_ctx, nu)
    zu = np.concatenate([inp["ctx_feats"].astype(F32), c_ctx, v_ctx,
                         ctx_emb.astype(F32)], 1) @ inp["W_u"].astype(F32) \
        + inp["b_u"].astype(F32)
    new_ctx = np.where(zu >= 0, zu, 0.1 * zu).astype(F32)

    return np.concatenate([new_clause, new_var, new_ctx], 0).astype(F32)
